# revision 40
# baseline (speedup 1.0000x reference)
"""Trainium2 Bass kernel for nn_Encoder_P: unwrap-diff-square front-end + 4 dilated
convs with dense concatenation, fused end-to-end on-chip.

Strategy (pure data parallel, 1 batch sample per NeuronCore, 8 cores):
  - The unwrap/diff/pad chain collapses: cumsum cancels in the diff, so
    sq[h] = wrap(p[h] - p[h-1])^2 (row 0 = 0), wrap(v) = v - 2*pi*k with
    k = (v>=pi) + (v>=3pi) - (v<=-pi) - (v<=-3pi).
  - Duplicate concat channels are folded into effective conv weights
    (conv3: 8->7 input planes, conv4: 20->15).
  - Each conv runs on TensorE as banded matmuls over the H (partition) axis:
    lhsT is a banded [128,128] H-shift matrix; rhs is the input plane tile
    [128 H, 516 Wpad]; PSUM accumulates over (ci, kw). Planes are stored as
    5 overlapping H-tiles (stride 107, halo 12) with zeroed W margins.

Production config (KCFG):
  - bf16 planes; conv1 runs as bf16 banded matmuls (weights must stay
    high-precision on the first stage); everything else runs as fp8e4m3
    DoubleRow matmuls over paired planes ([K,2,N] APs, K-doubled
    contraction, ~0.7x stream cycles vs bf16).
  - sq terms of conv2/3/4 use an error-feedback pair (sq_hi, sq_lo) with
    sq_lo = sq - fp8(sq) and the same band weight on both slots: the
    residual slot cancels the fp8 data-quantization error, so the sq path
    is MORE accurate than bf16 while running at DoubleRow rate.
    1120 matmul streams total: 40 bf16 + 1080 DoubleRow.
  - All out-channels (including the 17 duplicates) are written straight
    from SBUF with 2 batched dma_starts per channel (4-tile 3D AP + tail
    tile) - no DRAM->DRAM replication, saving ~8.5MB of HBM reads/core.
  - Param tables are split into per-conv DMA chunks so conv1's bands
    (0.26MB) land first instead of after the full 4.6MB of tables.
"""

import numpy as np

import concourse.bacc as bacc
import concourse.bass as bass
import concourse.mybir as mybir
import concourse.tile as tile
from concourse import bass_utils

F32 = mybir.dt.float32
BF16 = mybir.dt.bfloat16

H = 512
W = 512
S = 107          # tile stride in rows
HALO = 12        # halo rows above each tile
NT = 5           # number of H tiles
WPAD = 516       # 2 zero cols + 512 + 2 zero cols
WA = 528         # fp8 pair-slot stride (16B-aligned for DoubleRow APs)
P = 128
PI = float(np.pi)

# conv specs: (dil, pad_top, pad_left, KH, KW)
CONV_GEOM = [
    (1, 1, 1, 4, 4),   # conv1: 4x4 dil1, 'same' pad (1,2)
    (2, 2, 2, 3, 3),   # conv2: 3x3 dil2, pad (2,2)
    (3, 1, 1, 2, 2),   # conv3: 2x2 dil3, pad (1,2)
    (4, 0, 0, 1, 1),   # conv4: 1x1
]

PLANE_NAMES = (
    ["sq", "c1_0", "c1_1"]
    + [f"c2_{i}" for i in range(4)]
    + [f"c3_{i}" for i in range(8)]
)
CONV_INPUTS = [
    ["sq"],
    ["c1_0", "c1_1", "sq"],
    [f"c2_{i}" for i in range(4)] + ["c1_0", "c1_1", "sq"],
    [f"c3_{i}" for i in range(8)] + [f"c2_{i}" for i in range(4)]
    + ["c1_0", "c1_1", "sq"],
]
CONV_OUT = [2, 4, 8, 16]

# output channel -> source plane ("c4_o" channels handled separately)
CH_MAP = (
    [f"c4_{i}" for i in range(16)]
    + [f"c3_{i}" for i in range(8)]
    + [f"c2_{i}" for i in range(4)]
    + ["c1_0", "c1_1", "sq", "sq", "c1_0", "c1_1", "sq", "sq"]
    + [f"c2_{i}" for i in range(4)]
    + ["c1_0", "c1_1", "sq", "sq"]
    + ["c1_0", "c1_1", "sq", "sq"]
)

NSCAL = sum(
    CONV_OUT[c] * len(CONV_INPUTS[c]) * CONV_GEOM[c][3] * CONV_GEOM[c][4]
    for c in range(4)
)  # 604

# bf16 band table slots: conv1 (o*4+kw) 0-7, conv2-sq (o*3+kw) 8-19,
# conv3-sq (o*2+kw) 20-35, identity 36
NB_C1 = 0
NB_C2S = 8
NB_C3S = 20
NB_ID = 36
NB_TOT = 37

# fp8 pair-band table sections (P-col units; each pair-band = 2 P-cols)
F8_C2P = 0      # conv2 (c1_0,c1_1) pairs: (o*3+kw), 12 pair-bands
F8_C2S = 24     # conv2 sq-residual pairs: (o*3+kw), 12
F8_C2K = 48     # conv2 sq (kw0,kw1) pair-bands: (o), 4
F8_C3P = 56     # conv3 pairs: ((o*3+pj)*2+kw), 48
F8_C3K = 152    # conv3 sq (kw0,kw1) pair-bands: (o), 8
F8_C3S = 168    # conv3 sq-residual pairs: (o*2+kw), 16 (non-sqkw fallback)
F8_C4S = 200    # conv4 sq pairs: (o), 16
F8_TOT = 232


def _fold_weights(w1, w2, w3, w4):
    w3f = np.zeros((8, 7, 2, 2), np.float32)
    w3f[:, :6] = w3[:, :6]
    w3f[:, 6] = w3[:, 6] + w3[:, 7]
    w4f = np.zeros((16, 15, 1, 1), np.float32)
    w4f[:, :12] = w4[:, :12]
    w4f[:, 12] = w4[:, 12] + w4[:, 16]
    w4f[:, 13] = w4[:, 13] + w4[:, 17]
    w4f[:, 14] = w4[:, 14] + w4[:, 15] + w4[:, 18] + w4[:, 19]
    return [w1.astype(np.float32), w2.astype(np.float32), w3f, w4f]


# conv4 DoubleRow plane pairs (sq handled via its own residual pair)
PAIRS = [
    ("c3_0", "c3_1"), ("c3_2", "c3_3"), ("c3_4", "c3_5"), ("c3_6", "c3_7"),
    ("c2_0", "c2_1"), ("c2_2", "c2_3"), ("c1_0", "c1_1"),
]
# plane name -> column in folded w4f
W4COL = {f"c3_{i}": i for i in range(8)}
W4COL.update({f"c2_{i}": 8 + i for i in range(4)})
W4COL.update({"c1_0": 12, "c1_1": 13, "sq": 14})
PAIR_OF = {}
for _j, (_a, _b) in enumerate(PAIRS):
    PAIR_OF.setdefault(_a, []).append((_j, 0))
    PAIR_OF.setdefault(_b, []).append((_j, 1))


def _host_tables(inputs):
    """wtab [128, NSCAL], bias [128, 30], bands [128, NB_TOT*128] bf16,
    f8t [128, F8_TOT*128] fp8e4m3 host arrays."""
    import ml_dtypes

    wf = _fold_weights(inputs["w1"], inputs["w2"], inputs["w3"], inputs["w4"])
    scal = []
    for c in range(4):
        dil, pad_top, _, KH, KW = CONV_GEOM[c]
        for o in range(CONV_OUT[c]):
            for ci in range(len(CONV_INPUTS[c])):
                for kw in range(KW):
                    for kh in range(KH):
                        scal.append(wf[c][o, ci, kh, kw])
    assert len(scal) == NSCAL
    wtab = np.tile(np.asarray(scal, np.float32)[None, :], (P, 1))
    bias = np.concatenate(
        [inputs["b1"], inputs["b2"], inputs["b3"], inputs["b4"]]
    ).astype(np.float32)
    bias = np.tile(bias[None, :], (P, 1))

    def band(w_by_kh, deltas):
        b = np.zeros((P, P), np.float32)
        for w, d in zip(w_by_kh, deltas):
            b += w * np.eye(P, k=-d)
        return b

    d1 = [kh * 1 - 1 for kh in range(4)]
    d2 = [kh * 2 - 2 for kh in range(3)]
    d3 = [kh * 3 - 1 for kh in range(2)]

    bands = np.zeros((P, NB_TOT * P), np.float32)
    for o in range(2):
        for kw in range(4):
            j = NB_C1 + o * 4 + kw
            bands[:, j * P:(j + 1) * P] = band(wf[0][o, 0, :, kw], d1)
    for o in range(4):
        for kw in range(3):
            j = NB_C2S + o * 3 + kw
            bands[:, j * P:(j + 1) * P] = band(wf[1][o, 2, :, kw], d2)
    for o in range(8):
        for kw in range(2):
            j = NB_C3S + o * 2 + kw
            bands[:, j * P:(j + 1) * P] = band(wf[2][o, 6, :, kw], d3)
    bands[:, NB_ID * P:(NB_ID + 1) * P] = np.eye(P)
    bands = bands.astype(ml_dtypes.bfloat16)

    f8t = np.zeros((P, F8_TOT * P), np.float32)

    def put_pair(slot, b0, b1):
        f8t[:, slot * P:(slot + 1) * P] = b0
        f8t[:, (slot + 1) * P:(slot + 2) * P] = b1

    # conv2 (c1_0, c1_1) pairs
    for o in range(4):
        for kw in range(3):
            put_pair(F8_C2P + (o * 3 + kw) * 2,
                     band(wf[1][o, 0, :, kw], d2),
                     band(wf[1][o, 1, :, kw], d2))
    # conv2 sq residual pairs: same band both slots
    for o in range(4):
        for kw in range(3):
            b = band(wf[1][o, 2, :, kw], d2)
            put_pair(F8_C2S + (o * 3 + kw) * 2, b, b)
    # conv2 sq (kw0, kw1) pair-bands
    for o in range(4):
        put_pair(F8_C2K + o * 2,
                 band(wf[1][o, 2, :, 0], d2), band(wf[1][o, 2, :, 1], d2))
    # conv3 pairs
    ci3 = {"c2_0": 0, "c2_1": 1, "c2_2": 2, "c2_3": 3, "c1_0": 4, "c1_1": 5}
    for o in range(8):
        for pj, pr in enumerate(PAIRS[4:7]):
            for kw in range(2):
                put_pair(F8_C3P + ((o * 3 + pj) * 2 + kw) * 2,
                         band([wf[2][o, ci3[pr[0]], kh, kw] for kh in range(2)], d3),
                         band([wf[2][o, ci3[pr[1]], kh, kw] for kh in range(2)], d3))
    # conv3 sq (kw0, kw1) pair-bands
    for o in range(8):
        put_pair(F8_C3K + o * 2,
                 band(wf[2][o, 6, :, 0], d3), band(wf[2][o, 6, :, 1], d3))
    # conv3 sq residual pairs (non-sqkw fallback)
    for o in range(8):
        for kw in range(2):
            b = band(wf[2][o, 6, :, kw], d3)
            put_pair(F8_C3S + (o * 2 + kw) * 2, b, b)
    # conv4 sq residual pairs (diagonal)
    for o in range(16):
        b = wf[3][o, 14, 0, 0] * np.eye(P)
        put_pair(F8_C4S + o * 2, b, b)

    f8t = f8t.astype(ml_dtypes.float8_e4m3)
    return wtab, bias, bands, f8t


def build_nc(sq8=True, sqkw=True):
    # sqkw: True = kw-pair conv2+conv3 sq taps, 'c3' = conv3 only, False = off
    assert sq8 or not sqkw, "sqkw requires sq8"
    sqkw2 = sqkw is True
    sqkw3 = bool(sqkw)
    nc = bacc.Bacc("TRN2", target_bir_lowering=False, debug=False)
    FP8 = mybir.dt.float8e4
    o_dt = BF16

    p_dram = nc.dram_tensor("p", [H, W], F32, kind="ExternalInput")
    wtab_dram = nc.dram_tensor("wtab", [P, NSCAL], F32, kind="ExternalInput")
    bias_dram = nc.dram_tensor("bias", [P, 30], F32, kind="ExternalInput")
    bands_dram = nc.dram_tensor(
        "bands", [P, NB_TOT * P], BF16, kind="ExternalInput")
    f8t_dram = nc.dram_tensor(
        "f8t", [P, F8_TOT * P], FP8, kind="ExternalInput")
    out_dram = nc.dram_tensor("out", [48, H, W], o_dt, kind="ExternalOutput")

    # halo-grid planes (c3 planes live only as fp8 pairs + streamed out)
    halo_names = [nm for nm in PLANE_NAMES if not nm.startswith("c3_")]
    planes = {
        nm: nc.alloc_sbuf_tensor(f"pl_{nm}", [P, NT * WPAD], BF16)
        for nm in halo_names
    }
    # fp8 halo-grid pair-planes: dim1 = (tile, slot), dim2 = WA cols
    # (16B-aligned pair stride; data at [2, 514)).
    fp8_pairs = [
        nc.alloc_sbuf_tensor(f"f8p_{j}", [P, NT * 2, WA], FP8)
        for j in range(len(PAIRS))
    ]
    # sq fp8 lanes, 4 per tile: (hi, lo=residual, shift2, shift3). DoubleRow
    # rhs APs pick lane pairs by stride: (hi,lo) g=1, (hi,sh2) g=2 for
    # conv2's (kw0,kw1) stream, (hi,sh3) g=3 for conv3's (kw0,kw1) stream.
    sq_f8 = nc.alloc_sbuf_tensor("sq_f8", [P, NT * 4, WA], FP8)
    # conv4 DoubleRow pair-band table (DVE-built): (o, j) -> [128, 2, 128]
    c4b_sb = nc.alloc_sbuf_tensor("c4b_sb", [P, 16 * len(PAIRS) * 2 * P], FP8)
    wtab_sb = nc.alloc_sbuf_tensor("wtab_sb", [P, NSCAL], F32)
    bias_sb = nc.alloc_sbuf_tensor("bias_sb", [P, 30], F32)
    # with sq8 the conv2/3-sq fallback bands are never read: keep only
    # conv1's 8 slots + the identity (slot 8) on-chip
    nb_sb = 9 if sq8 else NB_TOT
    nb_id = 8 if sq8 else NB_ID
    bands_sb = nc.alloc_sbuf_tensor("bands_sb", [P, nb_sb * P], BF16)
    f8t_sb = nc.alloc_sbuf_tensor("f8t_sb", [P, F8_TOT * P], FP8)
    sqb_sb = (
        None if sq8
        else nc.alloc_sbuf_tensor("sqb_sb", [P, 16 * P], BF16)
    )

    def pslice(nm, t, c0, c1):
        return planes[nm][:, t * WPAD + c0: t * WPAD + c1]

    def f8pair(slot):
        return f8t_sb[:, slot * P:(slot + 2) * P].rearrange(
            "p (two m) -> p two m", two=2)

    DR = mybir.MatmulPerfMode.DoubleRow

    with tile.TileContext(nc) as tc:
        with (
            tc.tile_pool(name="io", bufs=4) as io_pool,
            tc.tile_pool(name="front", bufs=1) as fr_pool,
            tc.tile_pool(name="psum", bufs=8, space="PSUM") as psum_pool,
            tc.tile_pool(name="c4st", bufs=8) as c4_pool,
        ):
            # ---- parameter loads, split per conv and interleaved with the
            # front-end p loads (SP HWDGE drains FIFO: conv1's deps first) ----
            def param_dmas(t):
                # table loads ride the ACT HWDGE queue (nc.scalar) so they
                # never delay the p loads / output stores on SP's queue
                if t == -1:
                    nc.scalar.dma_start(
                        out=bands_sb[:, : NB_C2S * P],
                        in_=bands_dram[:, : NB_C2S * P])       # conv1 bands
                    nc.scalar.dma_start(out=bias_sb[:], in_=bias_dram[:])
                elif t == 0:
                    nc.scalar.dma_start(
                        out=f8t_sb[:, : F8_C3P * P],
                        in_=f8t_dram[:, : F8_C3P * P])         # conv2 sections
                elif t == 1:
                    nc.scalar.dma_start(out=wtab_sb[:], in_=wtab_dram[:])
                    if sq8:
                        nc.scalar.dma_start(
                            out=bands_sb[:, 8 * P: 9 * P],
                            in_=bands_dram[:, NB_ID * P:(NB_ID + 1) * P])
                    else:
                        nc.scalar.dma_start(
                            out=bands_sb[:, NB_C2S * P:],
                            in_=bands_dram[:, NB_C2S * P:])    # fallback+ident
                elif t == 2:
                    nc.scalar.dma_start(
                        out=f8t_sb[:, F8_C3P * P: F8_C4S * P],
                        in_=f8t_dram[:, F8_C3P * P: F8_C4S * P])  # conv3
                elif t == 3:
                    nc.scalar.dma_start(
                        out=f8t_sb[:, F8_C4S * P:],
                        in_=f8t_dram[:, F8_C4S * P:])          # conv4 sq
            param_dmas(-1)

            # zero fp8 pair tensors (W margins + unwritten slots read as 0)
            for t8 in fp8_pairs + [sq_f8]:
                nc.gpsimd.memset(
                    t8[:].rearrange("p a b -> p (a b)").bitcast(F32), 0.0)
            # zero W margins of all planes (written once)
            for nm in halo_names:
                for t in range(NT):
                    nc.gpsimd.memset(pslice(nm, t, 0, 2), 0.0)
                    nc.gpsimd.memset(pslice(nm, t, 514, 516), 0.0)

            def emit_chans(nm):
                """Write every output channel sourced from plane nm straight
                from SBUF: one 3D-AP dma_start covering tiles 0-3 + one for
                the tail tile."""
                for ch in [c for c in range(16, 48) if CH_MAP[c] == nm]:
                    src = planes[nm][HALO:HALO + S, :].rearrange(
                        "p (t w) -> p t w", w=WPAD)[:, 0:4, 2:514]
                    nc.sync.dma_start(
                        out=out_dram[ch, 0:4 * S, :].rearrange(
                            "(t r) w -> r t w", t=4),
                        in_=src)
                    nc.sync.dma_start(
                        out=out_dram[ch, 4 * S:H, :],
                        in_=planes[nm][HALO:HALO + H - 4 * S,
                                       4 * WPAD + 2: 4 * WPAD + 514])

            def to_pair(nm, t, src_ap):
                """DVE-convert a halo-grid [128, 512] AP into the fp8 pair
                slot(s) of plane nm at tile t (same partitions)."""
                for (pj, pi) in PAIR_OF.get(nm, []):
                    nc.vector.tensor_scalar(
                        fp8_pairs[pj][:, 2 * t + pi, 2:514],
                        src_ap, 1.0, None, mybir.AluOpType.mult,
                    )

            def conv_to_pairs(nm):
                for t in range(NT):
                    to_pair(nm, t, planes[nm][:, t * WPAD + 2: t * WPAD + 514])

            # warm-up matmuls: PE idles ~2.5us before conv1's deps land; a
            # short garbage stream (bands chunk as rhs, scratch psum) trips
            # the HAM activity window so conv1 runs at K=8/8 from the start
            warm_ps = psum_pool.tile([P, W], F32, tag="ps", name="warm")
            for wi in range(8):
                nc.tensor.matmul(
                    warm_ps, bands_sb[:, 0:P], bands_sb[:, 0:W],
                    start=True, stop=True,
                )

            # ---- front-end: sq (+ fp8 sq_hi / sq_lo residual pair) ----
            # A/B garbage regions are pre-zeroed so the out-of-image rows
            # compute v=0 -> sq=0, which is exactly the reference's zero pad.
            ao = mybir.AluOpType
            ABs = []
            for t in range(NT):
                p_lo = HALO if t == 0 else 0
                p_hi = H - (S * (NT - 1) - HALO) if t == NT - 1 else P
                n = p_hi - p_lo
                r_lo = S * t - HALO + p_lo
                A = io_pool.tile([P, W], F32, tag="A")
                B = io_pool.tile([P, W], F32, tag="B")
                if t == 0:
                    nc.gpsimd.memset(A[0:32, :], 0.0)
                    nc.gpsimd.memset(B[0:32, :], 0.0)
                if t == NT - 1:
                    nc.gpsimd.memset(A[96:P, :], 0.0)
                    nc.gpsimd.memset(B[96:P, :], 0.0)
                nc.sync.dma_start(out=A[p_lo:p_hi, :],
                                  in_=p_dram[r_lo: r_lo + n, :])
                if t == 0:
                    nc.sync.dma_start(
                        out=B[p_lo + 1:p_hi, :], in_=p_dram[0: n - 1, :])
                    nc.sync.dma_start(out=B[p_lo:p_lo + 1, :],
                                      in_=p_dram[0:1, :])
                else:
                    nc.sync.dma_start(
                        out=B[p_lo:p_hi, :],
                        in_=p_dram[r_lo - 1: r_lo - 1 + n, :])
                param_dmas(t)
                ABs.append((A, B))
            # wrap-classify chain, column-split across DVE (left half) and
            # GpSimd (right half) so the sq tiles conv1-4 wait on finish in
            # half the serial time. 8 ops per half:
            #   V = A - B;  K12 = (V>=pi) + (V>=3pi);  K34 = (V<=-pi)+(V<=-3pi)
            #   V += -2pi*(K12 - K34);  sq = V*V
            HW_ = W // 2
            for t in range(NT):
                A, B = ABs[t]
                V = fr_pool.tile([P, W], F32, tag="V")
                Ka = fr_pool.tile([P, W], F32, tag="Ka")
                Kb = fr_pool.tile([P, W], F32, tag="Kb")
                sq_dst = planes["sq"][:, t * WPAD + 2: t * WPAD + 514]
                Kc = fr_pool.tile([P, W], F32, tag="Kc")
                for eng, c0, c1 in ((nc.vector, 0, HW_), (nc.gpsimd, HW_, W)):
                    cs = slice(c0, c1)
                    eng.tensor_tensor(V[:, cs], A[:, cs], B[:, cs], ao.subtract)
                    eng.tensor_scalar(
                        Ka[:, cs], V[:, cs], 3 * PI, None, ao.is_ge)
                    eng.tensor_scalar(
                        Kb[:, cs], V[:, cs], -3 * PI, None, ao.is_le)
                    if eng is nc.vector:
                        # DVE: fused compare+add (Pool lacks this opcode)
                        eng.scalar_tensor_tensor(
                            Ka[:, cs], V[:, cs], PI, Ka[:, cs],
                            ao.is_ge, ao.add)
                        eng.scalar_tensor_tensor(
                            Kb[:, cs], V[:, cs], -PI, Kb[:, cs],
                            ao.is_le, ao.add)
                        eng.tensor_tensor(Ka[:, cs], Ka[:, cs], Kb[:, cs],
                                          ao.subtract)
                        eng.scalar_tensor_tensor(
                            V[:, cs], Ka[:, cs], -2 * PI, V[:, cs],
                            ao.mult, ao.add)
                    else:
                        eng.tensor_scalar(
                            Kc[:, cs], V[:, cs], PI, None, ao.is_ge)
                        eng.tensor_tensor(Ka[:, cs], Ka[:, cs], Kc[:, cs],
                                          ao.add)
                        eng.tensor_scalar(
                            Kc[:, cs], V[:, cs], -PI, None, ao.is_le)
                        eng.tensor_tensor(Kb[:, cs], Kb[:, cs], Kc[:, cs],
                                          ao.add)
                        eng.tensor_tensor(Ka[:, cs], Ka[:, cs], Kb[:, cs],
                                          ao.subtract)
                        eng.tensor_scalar(
                            Ka[:, cs], Ka[:, cs], -2 * PI, None, ao.mult)
                        eng.tensor_tensor(V[:, cs], V[:, cs], Ka[:, cs],
                                          ao.add)
                    eng.tensor_tensor(sq_dst[:, cs], V[:, cs], V[:, cs],
                                      ao.mult)
                    if sq8:
                        hi = sq_f8[:, 4 * t, 2 + c0:2 + c1]
                        lo = sq_f8[:, 4 * t + 1, 2 + c0:2 + c1]
                        eng.tensor_scalar(hi, sq_dst[:, cs], 1.0, None,
                                          ao.mult)
                        eng.tensor_tensor(lo, sq_dst[:, cs], hi, ao.subtract)
                    if sqkw2:
                        # sh2[c] = sq[c] (tail stays zero)
                        eng.tensor_scalar(
                            sq_f8[:, 4 * t + 2, c0:c1], sq_dst[:, cs],
                            1.0, None, ao.mult)
                if sqkw3:
                    # sh3[c] = sq[c+1] (tails stay zero)
                    nc.vector.tensor_scalar(
                        sq_f8[:, 4 * t + 3, 1:511],
                        planes["sq"][:, t * WPAD + 4: t * WPAD + 514],
                        1.0, None, ao.mult)
            emit_chans("sq")

            p_hi_last = H - (S * (NT - 1) - HALO)  # 96

            def edge_zero(nm):
                nc.gpsimd.memset(planes[nm][0:HALO, 0:WPAD], 0.0)
                nc.gpsimd.memset(
                    planes[nm][p_hi_last:P, (NT - 1) * WPAD: NT * WPAD], 0.0)

            # ---- conv1: bf16 banded matmuls from sq ----
            for o in range(2):
                psums = [
                    psum_pool.tile([P, W], F32, tag="ps", name=f"ps1_{o}_{t}")
                    for t in range(NT)
                ]
                for kw in range(4):
                    bandap = bands_sb[:, (NB_C1 + o * 4 + kw) * P:
                                      (NB_C1 + o * 4 + kw + 1) * P]
                    coff = 2 + kw * 1 - 1
                    for t in range(NT):
                        nc.tensor.matmul(
                            psums[t], bandap,
                            planes["sq"][:, t * WPAD + coff:
                                         t * WPAD + coff + W],
                            start=(kw == 0), stop=(kw == 3),
                        )
                bias_ap = bias_sb[:, o: o + 1]
                out_nm = f"c1_{o}"
                for t in range(NT):
                    nc.scalar.add(pslice(out_nm, t, 2, 514),
                                  psums[t][:], bias_ap)
                edge_zero(out_nm)
                emit_chans(out_nm)
                conv_to_pairs(out_nm)

            # ---- conv2: 3x3 dil2, DoubleRow (c1 pair + sq residual pair) ----
            for o in range(4):
                psums = [
                    psum_pool.tile([P, W], F32, tag="ps", name=f"ps2_{o}_{t}")
                    for t in range(NT)
                ]
                for kw in range(3):
                    lhsT = f8pair(F8_C2P + (o * 3 + kw) * 2)
                    coff = 2 + kw * 2 - 2
                    for t in range(NT):
                        rhs = fp8_pairs[6][:, 2 * t: 2 * t + 2,
                                           coff: coff + W]
                        nc.tensor.matmul(
                            psums[t], lhsT, rhs,
                            start=(kw == 0), stop=False, perf_mode=DR,
                        )
                if sqkw2:
                    # stream A: (kw0, kw1) via (hi, sh2) lanes at coff 0
                    lhsT = f8pair(F8_C2K + o * 2)
                    for t in range(NT):
                        rhs = sq_f8[:, 4 * t: 4 * t + 3: 2, 0:W]
                        nc.tensor.matmul(
                            psums[t], lhsT, rhs,
                            start=False, stop=False, perf_mode=DR,
                        )
                    # stream B: kw2 + residual via (hi, lo) lanes at coff 4
                    lhsT = f8pair(F8_C2S + (o * 3 + 2) * 2)
                    for t in range(NT):
                        rhs = sq_f8[:, 4 * t: 4 * t + 2, 4:4 + W]
                        nc.tensor.matmul(
                            psums[t], lhsT, rhs,
                            start=False, stop=True, perf_mode=DR,
                        )
                else:
                    for kw in range(3):
                        coff = 2 + kw * 2 - 2
                        if sq8:
                            lhsT = f8pair(F8_C2S + (o * 3 + kw) * 2)
                            for t in range(NT):
                                rhs = sq_f8[:, 4 * t: 4 * t + 2,
                                            coff: coff + W]
                                nc.tensor.matmul(
                                    psums[t], lhsT, rhs,
                                    start=False, stop=(kw == 2), perf_mode=DR,
                                )
                        else:
                            j = NB_C2S + o * 3 + kw
                            bandap = bands_sb[:, j * P:(j + 1) * P]
                            for t in range(NT):
                                nc.tensor.matmul(
                                    psums[t], bandap,
                                    planes["sq"][:, t * WPAD + coff:
                                                 t * WPAD + coff + W],
                                    start=False, stop=(kw == 2),
                                )
                bias_ap = bias_sb[:, 2 + o: 2 + o + 1]
                out_nm = f"c2_{o}"
                for t in range(NT):
                    nc.scalar.add(pslice(out_nm, t, 2, 514),
                                  psums[t][:], bias_ap)
                edge_zero(out_nm)
                emit_chans(out_nm)
                conv_to_pairs(out_nm)

            # conv4 DoubleRow pair-band table: diag(w) pairs. Built HERE (not
            # before conv1) so the 224 band ops don't sit ahead of conv2/3's
            # to_pair dependencies in the strict-FIFO engine queues; split
            # across DVE and GpSimd so neither engine stalls the pipeline.
            base4 = NSCAL - 240
            NPJ = len(PAIRS)
            ident_ap = bands_sb[:, nb_id * P:(nb_id + 1) * P]
            for o4 in range(16):
                eng = nc.vector if o4 % 2 == 0 else nc.gpsimd
                for pj, pr in enumerate(PAIRS):
                    for pi in (0, 1):
                        col = base4 + o4 * 15 + W4COL[pr[pi]]
                        k = ((o4 * NPJ + pj) * 2 + pi) * P
                        eng.tensor_scalar(
                            c4b_sb[:, k: k + P], ident_ap,
                            wtab_sb[:, col: col + 1], None, ao.mult,
                        )
                if not sq8:
                    colq = base4 + o4 * 15 + 14
                    eng.tensor_scalar(
                        sqb_sb[:, o4 * P:(o4 + 1) * P], ident_ap,
                        wtab_sb[:, colq: colq + 1], None, ao.mult,
                    )

            # ---- conv3: 2x2 dil3, DoubleRow; c3 evacs stream straight to
            # DRAM (+ fp8 pairs); no bf16 c3 planes are kept ----
            for o in range(8):
                psums = [
                    psum_pool.tile([P, W], F32, tag="ps", name=f"ps3_{o}_{t}")
                    for t in range(NT)
                ]
                first = True
                for pj3 in range(3):
                    for kw in range(2):
                        lhsT = f8pair(F8_C3P + ((o * 3 + pj3) * 2 + kw) * 2)
                        coff = 2 + kw * 3 - 1
                        for t in range(NT):
                            rhs = fp8_pairs[4 + pj3][:, 2 * t: 2 * t + 2,
                                                     coff: coff + W]
                            nc.tensor.matmul(
                                psums[t], lhsT, rhs,
                                start=first, stop=False, perf_mode=DR,
                            )
                        first = False
                if sqkw3:
                    # (kw0, kw1) in one stream via (hi, sh3) lanes at coff 1
                    lhsT = f8pair(F8_C3K + o * 2)
                    for t in range(NT):
                        rhs = sq_f8[:, 4 * t: 4 * t + 4: 3, 1:1 + W]
                        nc.tensor.matmul(
                            psums[t], lhsT, rhs,
                            start=False, stop=True, perf_mode=DR,
                        )
                else:
                    for kw in range(2):
                        coff = 2 + kw * 3 - 1
                        if sq8:
                            lhsT = f8pair(F8_C3S + (o * 2 + kw) * 2)
                            for t in range(NT):
                                rhs = sq_f8[:, 4 * t: 4 * t + 2,
                                            coff: coff + W]
                                nc.tensor.matmul(
                                    psums[t], lhsT, rhs,
                                    start=False, stop=(kw == 1), perf_mode=DR,
                                )
                        else:
                            j = NB_C3S + o * 2 + kw
                            bandap = bands_sb[:, j * P:(j + 1) * P]
                            for t in range(NT):
                                nc.tensor.matmul(
                                    psums[t], bandap,
                                    planes["sq"][:, t * WPAD + coff:
                                                 t * WPAD + coff + W],
                                    start=False, stop=(kw == 1),
                                )
                bias_ap = bias_sb[:, 6 + o: 6 + o + 1]
                # fp8 pairs for conv4 convert straight from PSUM (DVE does
                # bias-add + fp8 round) so conv4's inputs never wait on the
                # store scratch buffers or the out-DMA queue
                for t in range(NT):
                    for (pj, pi) in PAIR_OF[f"c3_{o}"]:
                        nc.vector.tensor_scalar(
                            fp8_pairs[pj][:, 2 * t + pi, 2:514],
                            psums[t][:], bias_ap, None, ao.add,
                        )
                st4 = c4_pool.tile([P, 4 * W], o_dt, tag="c4w")
                for t in range(4):
                    nc.scalar.add(st4[:, t * W:(t + 1) * W],
                                  psums[t][:], bias_ap)
                nc.sync.dma_start(
                    out=out_dram[16 + o, 0:4 * S, :].rearrange(
                        "(t r) w -> r t w", t=4),
                    in_=st4[HALO:HALO + S, :].rearrange(
                        "p (t w) -> p t w", t=4),
                )
                st = c4_pool.tile([P, W], o_dt, tag="c4")
                nc.scalar.add(st[:], psums[4][:], bias_ap)
                nc.sync.dma_start(
                    out=out_dram[16 + o, 4 * S:H, :],
                    in_=st[HALO:HALO + H - 4 * S, :],
                )

            # ---- conv4: 1x1, DoubleRow over 7 plane pairs + sq pair ----
            for o in range(16):
                psums = [
                    psum_pool.tile([P, W], F32, tag="ps", name=f"ps4_{o}_{t}")
                    for t in range(NT)
                ]
                for pj in range(NPJ):
                    k = (o * NPJ + pj) * 2 * P
                    lhsT = c4b_sb[:, k: k + 2 * P].rearrange(
                        "p (two m) -> p two m", two=2)
                    for t in range(NT):
                        rhs = fp8_pairs[pj][:, 2 * t: 2 * t + 2, 2:514]
                        nc.tensor.matmul(
                            psums[t], lhsT, rhs,
                            start=(pj == 0), stop=False, perf_mode=DR,
                        )
                if sq8:
                    lhsT = f8pair(F8_C4S + o * 2)
                    for t in range(NT):
                        rhs = sq_f8[:, 4 * t: 4 * t + 2, 2:514]
                        nc.tensor.matmul(
                            psums[t], lhsT, rhs,
                            start=False, stop=True, perf_mode=DR,
                        )
                else:
                    for t in range(NT):
                        nc.tensor.matmul(
                            psums[t], sqb_sb[:, o * P:(o + 1) * P],
                            planes["sq"][:, t * WPAD + 2: t * WPAD + 514],
                            start=False, stop=True,
                        )
                bias_ap = bias_sb[:, 14 + o: 14 + o + 1]
                if o == 15:
                    # last channel: per-tile stores with evacs spread across
                    # Act/DVE/GpSimd so the kernel tail is ~one evac+DMA long
                    for t in range(NT):
                        st = c4_pool.tile([P, W], o_dt, tag="c4")
                        if t % 2 == 0:
                            nc.scalar.add(st[:], psums[t][:], bias_ap)
                        else:
                            nc.vector.tensor_scalar(
                                st[:], psums[t][:], bias_ap, None, ao.add)
                        rows = S if t < NT - 1 else H - S * (NT - 1)
                        nc.sync.dma_start(
                            out=out_dram[o, S * t: S * t + rows, :],
                            in_=st[HALO:HALO + rows, :],
                        )
                    continue
                st4 = c4_pool.tile([P, 4 * W], o_dt, tag="c4w")
                for t in range(4):
                    nc.scalar.add(st4[:, t * W:(t + 1) * W],
                                  psums[t][:], bias_ap)
                nc.sync.dma_start(
                    out=out_dram[o, 0:4 * S, :].rearrange(
                        "(t r) w -> r t w", t=4),
                    in_=st4[HALO:HALO + S, :].rearrange(
                        "p (t w) -> p t w", t=4),
                )
                st = c4_pool.tile([P, W], o_dt, tag="c4")
                nc.scalar.add(st[:], psums[4][:], bias_ap)
                nc.sync.dma_start(
                    out=out_dram[o, 4 * S:H, :],
                    in_=st[HALO:HALO + H - 4 * S, :],
                )

    nc.compile()
    return nc


_NC_CACHE = None

KCFG = dict(sq8=True, sqkw='c3')


def _get_nc():
    global _NC_CACHE
    if _NC_CACHE is None:
        _NC_CACHE = build_nc(**KCFG)
    return _NC_CACHE


def _in_maps(inputs, n_cores):
    wtab, bias, bands, f8t = _host_tables(inputs)
    feat = inputs["feature_in"].astype(np.float32)  # [8,1,512,512]
    return [
        {"p": feat[b, 0], "wtab": wtab, "bias": bias,
         "bands": bands, "f8t": f8t}
        for b in range(n_cores)
    ]


def _run(inputs, trace=False):
    inputs = {k: np.asarray(v) for k, v in inputs.items()}
    nc = _get_nc()
    n_cores = inputs["feature_in"].shape[0]
    in_maps = _in_maps(inputs, n_cores)
    res = bass_utils.run_bass_kernel_spmd(
        nc, in_maps, core_ids=list(range(n_cores)), trace=trace
    )
    out = np.stack([res.results[b]["out"] for b in range(n_cores)], axis=0)
    return out.astype(np.float32), res


def kernel(**inputs):
    return _run(inputs, trace=False)[0]


# revision 42
# speedup vs baseline: 1.1818x; 1.1818x over previous
"""Trainium2 Bass kernel for nn_Encoder_P: unwrap-diff-square front-end + 4 dilated
convs with dense concatenation, fused end-to-end on-chip.

Strategy (pure data parallel, 1 batch sample per NeuronCore, 8 cores):
  - The unwrap/diff/pad chain collapses: cumsum cancels in the diff, so
    sq[h] = wrap(p[h] - p[h-1])^2 (row 0 = 0), wrap(v) = v - 2*pi*k with
    k = (v>=pi) + (v>=3pi) - (v<=-pi) - (v<=-3pi).
  - Duplicate concat channels are folded into effective conv weights
    (conv3: 8->7 input planes, conv4: 20->15).
  - Each conv runs on TensorE as banded matmuls over the H (partition) axis:
    lhsT is a banded [128,128] H-shift matrix; rhs is the input plane tile
    [128 H, 516 Wpad]; PSUM accumulates over (ci, kw). Planes are stored as
    5 overlapping H-tiles (stride 107, halo 12) with zeroed W margins.

Production config (KCFG = sq8=True, sqkw='c3'; HW rel err 1.545e-02):
  - bf16 planes; conv1 runs as bf16 banded matmuls (weights must stay
    high-precision on the first stage); everything else runs as fp8e4m3
    DoubleRow matmuls over paired planes ([K,2,N] APs, K-doubled
    contraction, ~0.7x stream cycles vs bf16).
  - sq terms of conv2/4 use an error-feedback pair (sq_hi, sq_lo) with
    sq_lo = sq - fp8(sq) and the same band weight on both slots: the
    residual slot cancels the fp8 data-quantization error. conv3's two
    sq kw-taps are instead packed into ONE DoubleRow stream via a
    pre-shifted lane (hi, sh3) - stride-3 lane AP - trading the residual
    for a whole stream. 1088 matmul streams total: 48 bf16 (conv1 +
    8 HAM-warmup) + 1040 DoubleRow -> ~100us of TensorE stream time.
  - All out-channels (including the 17 duplicates) are written straight
    from SBUF with 2 batched dma_starts per channel (4-tile 3D AP + tail
    tile) - no DRAM->DRAM replication; ~31MB HBM/core total vs 39MB.
  - Scheduling: param tables ride the ACT HWDGE queue (SP keeps the
    p loads + stores); the wrap-classify front-end is column-split
    across DVE and GpSimd (GpSimd lacks scalar_tensor_tensor, so its
    half runs an unfused chain); the conv4 pair-band build (224 diag
    bands) runs between conv2 and conv3, split DVE/GpSimd; c3 psums
    convert to fp8 pairs directly from PSUM so conv4 never waits on the
    store scratch; the last channel's evacs alternate Act/DVE to cut
    the kernel tail.
"""

import numpy as np

import concourse.bacc as bacc
import concourse.bass as bass
import concourse.mybir as mybir
import concourse.tile as tile
from concourse import bass_utils

F32 = mybir.dt.float32
BF16 = mybir.dt.bfloat16

H = 512
W = 512
S = 107          # tile stride in rows
HALO = 12        # halo rows above each tile
NT = 5           # number of H tiles
WPAD = 516       # 2 zero cols + 512 + 2 zero cols
WA = 528         # fp8 pair-slot stride (16B-aligned for DoubleRow APs)
P = 128
PI = float(np.pi)

# conv specs: (dil, pad_top, pad_left, KH, KW)
CONV_GEOM = [
    (1, 1, 1, 4, 4),   # conv1: 4x4 dil1, 'same' pad (1,2)
    (2, 2, 2, 3, 3),   # conv2: 3x3 dil2, pad (2,2)
    (3, 1, 1, 2, 2),   # conv3: 2x2 dil3, pad (1,2)
    (4, 0, 0, 1, 1),   # conv4: 1x1
]

PLANE_NAMES = (
    ["sq", "c1_0", "c1_1"]
    + [f"c2_{i}" for i in range(4)]
    + [f"c3_{i}" for i in range(8)]
)
CONV_INPUTS = [
    ["sq"],
    ["c1_0", "c1_1", "sq"],
    [f"c2_{i}" for i in range(4)] + ["c1_0", "c1_1", "sq"],
    [f"c3_{i}" for i in range(8)] + [f"c2_{i}" for i in range(4)]
    + ["c1_0", "c1_1", "sq"],
]
CONV_OUT = [2, 4, 8, 16]

# output channel -> source plane ("c4_o" channels handled separately)
CH_MAP = (
    [f"c4_{i}" for i in range(16)]
    + [f"c3_{i}" for i in range(8)]
    + [f"c2_{i}" for i in range(4)]
    + ["c1_0", "c1_1", "sq", "sq", "c1_0", "c1_1", "sq", "sq"]
    + [f"c2_{i}" for i in range(4)]
    + ["c1_0", "c1_1", "sq", "sq"]
    + ["c1_0", "c1_1", "sq", "sq"]
)

NSCAL = sum(
    CONV_OUT[c] * len(CONV_INPUTS[c]) * CONV_GEOM[c][3] * CONV_GEOM[c][4]
    for c in range(4)
)  # 604

# bf16 band table slots: conv1 (o*4+kw) 0-7, conv2-sq (o*3+kw) 8-19,
# conv3-sq (o*2+kw) 20-35, identity 36
NB_C1 = 0
NB_C2S = 8
NB_C3S = 20
NB_ID = 36
NB_TOT = 37

# fp8 pair-band table sections (P-col units; each pair-band = 2 P-cols)
F8_C2P = 0      # conv2 (c1_0,c1_1) pairs: (o*3+kw), 12 pair-bands
F8_C2S = 24     # conv2 sq-residual pairs: (o*3+kw), 12
F8_C2K = 48     # conv2 sq (kw0,kw1) pair-bands: (o), 4
F8_C3P = 56     # conv3 pairs: ((o*3+pj)*2+kw), 48
F8_C3K = 152    # conv3 sq (kw0,kw1) pair-bands: (o), 8
F8_C3S = 168    # conv3 sq-residual pairs: (o*2+kw), 16 (non-sqkw fallback)
F8_C4S = 200    # conv4 sq pairs: (o), 16
F8_TOT = 232


def _fold_weights(w1, w2, w3, w4):
    w3f = np.zeros((8, 7, 2, 2), np.float32)
    w3f[:, :6] = w3[:, :6]
    w3f[:, 6] = w3[:, 6] + w3[:, 7]
    w4f = np.zeros((16, 15, 1, 1), np.float32)
    w4f[:, :12] = w4[:, :12]
    w4f[:, 12] = w4[:, 12] + w4[:, 16]
    w4f[:, 13] = w4[:, 13] + w4[:, 17]
    w4f[:, 14] = w4[:, 14] + w4[:, 15] + w4[:, 18] + w4[:, 19]
    return [w1.astype(np.float32), w2.astype(np.float32), w3f, w4f]


# conv4 DoubleRow plane pairs (sq handled via its own residual pair)
PAIRS = [
    ("c3_0", "c3_1"), ("c3_2", "c3_3"), ("c3_4", "c3_5"), ("c3_6", "c3_7"),
    ("c2_0", "c2_1"), ("c2_2", "c2_3"), ("c1_0", "c1_1"),
]
# plane name -> column in folded w4f
W4COL = {f"c3_{i}": i for i in range(8)}
W4COL.update({f"c2_{i}": 8 + i for i in range(4)})
W4COL.update({"c1_0": 12, "c1_1": 13, "sq": 14})
PAIR_OF = {}
for _j, (_a, _b) in enumerate(PAIRS):
    PAIR_OF.setdefault(_a, []).append((_j, 0))
    PAIR_OF.setdefault(_b, []).append((_j, 1))


def _host_tables(inputs):
    """wtab [128, NSCAL], bias [128, 30], bands [128, NB_TOT*128] bf16,
    f8t [128, F8_TOT*128] fp8e4m3 host arrays."""
    import ml_dtypes

    wf = _fold_weights(inputs["w1"], inputs["w2"], inputs["w3"], inputs["w4"])
    scal = []
    for c in range(4):
        dil, pad_top, _, KH, KW = CONV_GEOM[c]
        for o in range(CONV_OUT[c]):
            for ci in range(len(CONV_INPUTS[c])):
                for kw in range(KW):
                    for kh in range(KH):
                        scal.append(wf[c][o, ci, kh, kw])
    assert len(scal) == NSCAL
    wtab = np.tile(np.asarray(scal, np.float32)[None, :], (P, 1))
    bias = np.concatenate(
        [inputs["b1"], inputs["b2"], inputs["b3"], inputs["b4"]]
    ).astype(np.float32)
    bias = np.tile(bias[None, :], (P, 1))

    def band(w_by_kh, deltas):
        b = np.zeros((P, P), np.float32)
        for w, d in zip(w_by_kh, deltas):
            b += w * np.eye(P, k=-d)
        return b

    d1 = [kh * 1 - 1 for kh in range(4)]
    d2 = [kh * 2 - 2 for kh in range(3)]
    d3 = [kh * 3 - 1 for kh in range(2)]

    bands = np.zeros((P, NB_TOT * P), np.float32)
    for o in range(2):
        for kw in range(4):
            j = NB_C1 + o * 4 + kw
            bands[:, j * P:(j + 1) * P] = band(wf[0][o, 0, :, kw], d1)
    for o in range(4):
        for kw in range(3):
            j = NB_C2S + o * 3 + kw
            bands[:, j * P:(j + 1) * P] = band(wf[1][o, 2, :, kw], d2)
    for o in range(8):
        for kw in range(2):
            j = NB_C3S + o * 2 + kw
            bands[:, j * P:(j + 1) * P] = band(wf[2][o, 6, :, kw], d3)
    bands[:, NB_ID * P:(NB_ID + 1) * P] = np.eye(P)
    bands = bands.astype(ml_dtypes.bfloat16)

    f8t = np.zeros((P, F8_TOT * P), np.float32)

    def put_pair(slot, b0, b1):
        f8t[:, slot * P:(slot + 1) * P] = b0
        f8t[:, (slot + 1) * P:(slot + 2) * P] = b1

    # conv2 (c1_0, c1_1) pairs
    for o in range(4):
        for kw in range(3):
            put_pair(F8_C2P + (o * 3 + kw) * 2,
                     band(wf[1][o, 0, :, kw], d2),
                     band(wf[1][o, 1, :, kw], d2))
    # conv2 sq residual pairs: same band both slots
    for o in range(4):
        for kw in range(3):
            b = band(wf[1][o, 2, :, kw], d2)
            put_pair(F8_C2S + (o * 3 + kw) * 2, b, b)
    # conv2 sq (kw0, kw1) pair-bands
    for o in range(4):
        put_pair(F8_C2K + o * 2,
                 band(wf[1][o, 2, :, 0], d2), band(wf[1][o, 2, :, 1], d2))
    # conv3 pairs
    ci3 = {"c2_0": 0, "c2_1": 1, "c2_2": 2, "c2_3": 3, "c1_0": 4, "c1_1": 5}
    for o in range(8):
        for pj, pr in enumerate(PAIRS[4:7]):
            for kw in range(2):
                put_pair(F8_C3P + ((o * 3 + pj) * 2 + kw) * 2,
                         band([wf[2][o, ci3[pr[0]], kh, kw] for kh in range(2)], d3),
                         band([wf[2][o, ci3[pr[1]], kh, kw] for kh in range(2)], d3))
    # conv3 sq (kw0, kw1) pair-bands
    for o in range(8):
        put_pair(F8_C3K + o * 2,
                 band(wf[2][o, 6, :, 0], d3), band(wf[2][o, 6, :, 1], d3))
    # conv3 sq residual pairs (non-sqkw fallback)
    for o in range(8):
        for kw in range(2):
            b = band(wf[2][o, 6, :, kw], d3)
            put_pair(F8_C3S + (o * 2 + kw) * 2, b, b)
    # conv4 sq residual pairs (diagonal)
    for o in range(16):
        b = wf[3][o, 14, 0, 0] * np.eye(P)
        put_pair(F8_C4S + o * 2, b, b)

    f8t = f8t.astype(ml_dtypes.float8_e4m3)
    return wtab, bias, bands, f8t


def build_nc(sq8=True, sqkw=True):
    # sqkw: True = kw-pair conv2+conv3 sq taps, 'c3' = conv3 only, False = off
    assert sq8 or not sqkw, "sqkw requires sq8"
    sqkw2 = sqkw is True
    sqkw3 = bool(sqkw)
    nc = bacc.Bacc("TRN2", target_bir_lowering=False, debug=False)
    FP8 = mybir.dt.float8e4
    o_dt = BF16

    p_dram = nc.dram_tensor("p", [H, W], F32, kind="ExternalInput")
    wtab_dram = nc.dram_tensor("wtab", [P, NSCAL], F32, kind="ExternalInput")
    bias_dram = nc.dram_tensor("bias", [P, 30], F32, kind="ExternalInput")
    bands_dram = nc.dram_tensor(
        "bands", [P, NB_TOT * P], BF16, kind="ExternalInput")
    f8t_dram = nc.dram_tensor(
        "f8t", [P, F8_TOT * P], FP8, kind="ExternalInput")
    out_dram = nc.dram_tensor("out", [48, H, W], o_dt, kind="ExternalOutput")

    # halo-grid planes (c3 planes live only as fp8 pairs + streamed out)
    halo_names = [nm for nm in PLANE_NAMES if not nm.startswith("c3_")]
    planes = {
        nm: nc.alloc_sbuf_tensor(f"pl_{nm}", [P, NT * WPAD], BF16)
        for nm in halo_names
    }
    # fp8 halo-grid pair-planes: dim1 = (tile, slot), dim2 = WA cols
    # (16B-aligned pair stride; data at [2, 514)).
    fp8_pairs = [
        nc.alloc_sbuf_tensor(f"f8p_{j}", [P, NT * 2, WA], FP8)
        for j in range(len(PAIRS))
    ]
    # sq fp8 lanes, 4 per tile: (hi, lo=residual, shift2, shift3). DoubleRow
    # rhs APs pick lane pairs by stride: (hi,lo) g=1, (hi,sh2) g=2 for
    # conv2's (kw0,kw1) stream, (hi,sh3) g=3 for conv3's (kw0,kw1) stream.
    sq_f8 = nc.alloc_sbuf_tensor("sq_f8", [P, NT * 4, WA], FP8)
    # conv4 DoubleRow pair-band table (DVE-built): (o, j) -> [128, 2, 128]
    c4b_sb = nc.alloc_sbuf_tensor("c4b_sb", [P, 16 * len(PAIRS) * 2 * P], FP8)
    wtab_sb = nc.alloc_sbuf_tensor("wtab_sb", [P, NSCAL], F32)
    bias_sb = nc.alloc_sbuf_tensor("bias_sb", [P, 30], F32)
    # with sq8 the conv2/3-sq fallback bands are never read: keep only
    # conv1's 8 slots + the identity (slot 8) on-chip
    nb_sb = 9 if sq8 else NB_TOT
    nb_id = 8 if sq8 else NB_ID
    bands_sb = nc.alloc_sbuf_tensor("bands_sb", [P, nb_sb * P], BF16)
    f8t_sb = nc.alloc_sbuf_tensor("f8t_sb", [P, F8_TOT * P], FP8)
    sqb_sb = (
        None if sq8
        else nc.alloc_sbuf_tensor("sqb_sb", [P, 16 * P], BF16)
    )

    def pslice(nm, t, c0, c1):
        return planes[nm][:, t * WPAD + c0: t * WPAD + c1]

    def f8pair(slot):
        return f8t_sb[:, slot * P:(slot + 2) * P].rearrange(
            "p (two m) -> p two m", two=2)

    DR = mybir.MatmulPerfMode.DoubleRow

    with tile.TileContext(nc) as tc:
        with (
            tc.tile_pool(name="io", bufs=4) as io_pool,
            tc.tile_pool(name="front", bufs=1) as fr_pool,
            tc.tile_pool(name="psum", bufs=8, space="PSUM") as psum_pool,
            tc.tile_pool(name="c4st", bufs=8) as c4_pool,
        ):
            # ---- parameter loads, split per conv and interleaved with the
            # front-end p loads (SP HWDGE drains FIFO: conv1's deps first) ----
            def param_dmas(t):
                # table loads ride the ACT HWDGE queue (nc.scalar) so they
                # never delay the p loads / output stores on SP's queue
                if t == -1:
                    nc.scalar.dma_start(
                        out=bands_sb[:, : NB_C2S * P],
                        in_=bands_dram[:, : NB_C2S * P])       # conv1 bands
                    nc.scalar.dma_start(out=bias_sb[:], in_=bias_dram[:])
                elif t == 0:
                    c2_hi = F8_C3P if sqkw2 else F8_C2K
                    nc.scalar.dma_start(
                        out=f8t_sb[:, : c2_hi * P],
                        in_=f8t_dram[:, : c2_hi * P])          # conv2 sections
                elif t == 1:
                    nc.scalar.dma_start(out=wtab_sb[:], in_=wtab_dram[:])
                    if sq8:
                        nc.scalar.dma_start(
                            out=bands_sb[:, 8 * P: 9 * P],
                            in_=bands_dram[:, NB_ID * P:(NB_ID + 1) * P])
                    else:
                        nc.scalar.dma_start(
                            out=bands_sb[:, NB_C2S * P:],
                            in_=bands_dram[:, NB_C2S * P:])    # fallback+ident
                elif t == 2:
                    c3_hi = F8_C4S if not sqkw3 else F8_C3S
                    nc.scalar.dma_start(
                        out=f8t_sb[:, F8_C3P * P: c3_hi * P],
                        in_=f8t_dram[:, F8_C3P * P: c3_hi * P])   # conv3
                elif t == 3:
                    nc.scalar.dma_start(
                        out=f8t_sb[:, F8_C4S * P:],
                        in_=f8t_dram[:, F8_C4S * P:])          # conv4 sq
            param_dmas(-1)

            # zero fp8 pair tensors (W margins + unwritten slots read as 0)
            for t8 in fp8_pairs + [sq_f8]:
                nc.gpsimd.memset(
                    t8[:].rearrange("p a b -> p (a b)").bitcast(F32), 0.0)
            # zero W margins of all planes (written once)
            for nm in halo_names:
                for t in range(NT):
                    nc.gpsimd.memset(pslice(nm, t, 0, 2), 0.0)
                    nc.gpsimd.memset(pslice(nm, t, 514, 516), 0.0)

            def emit_chans(nm):
                """Write every output channel sourced from plane nm straight
                from SBUF: one 3D-AP dma_start covering tiles 0-3 + one for
                the tail tile."""
                for ch in [c for c in range(16, 48) if CH_MAP[c] == nm]:
                    src = planes[nm][HALO:HALO + S, :].rearrange(
                        "p (t w) -> p t w", w=WPAD)[:, 0:4, 2:514]
                    nc.sync.dma_start(
                        out=out_dram[ch, 0:4 * S, :].rearrange(
                            "(t r) w -> r t w", t=4),
                        in_=src)
                    nc.sync.dma_start(
                        out=out_dram[ch, 4 * S:H, :],
                        in_=planes[nm][HALO:HALO + H - 4 * S,
                                       4 * WPAD + 2: 4 * WPAD + 514])

            def to_pair(nm, t, src_ap):
                """DVE-convert a halo-grid [128, 512] AP into the fp8 pair
                slot(s) of plane nm at tile t (same partitions)."""
                for (pj, pi) in PAIR_OF.get(nm, []):
                    nc.vector.tensor_scalar(
                        fp8_pairs[pj][:, 2 * t + pi, 2:514],
                        src_ap, 1.0, None, mybir.AluOpType.mult,
                    )

            def conv_to_pairs(nm):
                for t in range(NT):
                    to_pair(nm, t, planes[nm][:, t * WPAD + 2: t * WPAD + 514])

            # warm-up matmuls: PE idles ~2.5us before conv1's deps land; a
            # short garbage stream (bands chunk as rhs, scratch psum) trips
            # the HAM activity window so conv1 runs at K=8/8 from the start
            warm_ps = psum_pool.tile([P, W], F32, tag="ps", name="warm")
            for wi in range(8):
                nc.tensor.matmul(
                    warm_ps, bands_sb[:, 0:P], bands_sb[:, 0:W],
                    start=True, stop=True,
                )

            # ---- front-end: sq (+ fp8 sq_hi / sq_lo residual pair) ----
            # A/B garbage regions are pre-zeroed so the out-of-image rows
            # compute v=0 -> sq=0, which is exactly the reference's zero pad.
            ao = mybir.AluOpType
            ABs = []
            for t in range(NT):
                p_lo = HALO if t == 0 else 0
                p_hi = H - (S * (NT - 1) - HALO) if t == NT - 1 else P
                n = p_hi - p_lo
                r_lo = S * t - HALO + p_lo
                A = io_pool.tile([P, W], F32, tag="A")
                B = io_pool.tile([P, W], F32, tag="B")
                if t == 0:
                    nc.gpsimd.memset(A[0:32, :], 0.0)
                    nc.gpsimd.memset(B[0:32, :], 0.0)
                if t == NT - 1:
                    nc.gpsimd.memset(A[96:P, :], 0.0)
                    nc.gpsimd.memset(B[96:P, :], 0.0)
                nc.sync.dma_start(out=A[p_lo:p_hi, :],
                                  in_=p_dram[r_lo: r_lo + n, :])
                if t == 0:
                    nc.sync.dma_start(
                        out=B[p_lo + 1:p_hi, :], in_=p_dram[0: n - 1, :])
                    nc.sync.dma_start(out=B[p_lo:p_lo + 1, :],
                                      in_=p_dram[0:1, :])
                else:
                    nc.sync.dma_start(
                        out=B[p_lo:p_hi, :],
                        in_=p_dram[r_lo - 1: r_lo - 1 + n, :])
                param_dmas(t)
                ABs.append((A, B))
            # wrap-classify chain, column-split across DVE (left half) and
            # GpSimd (right half) so the sq tiles conv1-4 wait on finish in
            # half the serial time. 8 ops per half:
            #   V = A - B;  K12 = (V>=pi) + (V>=3pi);  K34 = (V<=-pi)+(V<=-3pi)
            #   V += -2pi*(K12 - K34);  sq = V*V
            HW_ = W // 2
            for t in range(NT):
                A, B = ABs[t]
                V = fr_pool.tile([P, W], F32, tag="V")
                Ka = fr_pool.tile([P, W], F32, tag="Ka")
                Kb = fr_pool.tile([P, W], F32, tag="Kb")
                sq_dst = planes["sq"][:, t * WPAD + 2: t * WPAD + 514]
                Kc = fr_pool.tile([P, W], F32, tag="Kc")
                for eng, c0, c1 in ((nc.vector, 0, HW_), (nc.gpsimd, HW_, W)):
                    cs = slice(c0, c1)
                    eng.tensor_tensor(V[:, cs], A[:, cs], B[:, cs], ao.subtract)
                    eng.tensor_scalar(
                        Ka[:, cs], V[:, cs], 3 * PI, None, ao.is_ge)
                    eng.tensor_scalar(
                        Kb[:, cs], V[:, cs], -3 * PI, None, ao.is_le)
                    if eng is nc.vector:
                        # DVE: fused compare+add (Pool lacks this opcode)
                        eng.scalar_tensor_tensor(
                            Ka[:, cs], V[:, cs], PI, Ka[:, cs],
                            ao.is_ge, ao.add)
                        eng.scalar_tensor_tensor(
                            Kb[:, cs], V[:, cs], -PI, Kb[:, cs],
                            ao.is_le, ao.add)
                        eng.tensor_tensor(Ka[:, cs], Ka[:, cs], Kb[:, cs],
                                          ao.subtract)
                        eng.scalar_tensor_tensor(
                            V[:, cs], Ka[:, cs], -2 * PI, V[:, cs],
                            ao.mult, ao.add)
                    else:
                        eng.tensor_scalar(
                            Kc[:, cs], V[:, cs], PI, None, ao.is_ge)
                        eng.tensor_tensor(Ka[:, cs], Ka[:, cs], Kc[:, cs],
                                          ao.add)
                        eng.tensor_scalar(
                            Kc[:, cs], V[:, cs], -PI, None, ao.is_le)
                        eng.tensor_tensor(Kb[:, cs], Kb[:, cs], Kc[:, cs],
                                          ao.add)
                        eng.tensor_tensor(Ka[:, cs], Ka[:, cs], Kb[:, cs],
                                          ao.subtract)
                        eng.tensor_scalar(
                            Ka[:, cs], Ka[:, cs], -2 * PI, None, ao.mult)
                        eng.tensor_tensor(V[:, cs], V[:, cs], Ka[:, cs],
                                          ao.add)
                    eng.tensor_tensor(sq_dst[:, cs], V[:, cs], V[:, cs],
                                      ao.mult)
                    if sq8:
                        hi = sq_f8[:, 4 * t, 2 + c0:2 + c1]
                        lo = sq_f8[:, 4 * t + 1, 2 + c0:2 + c1]
                        eng.tensor_scalar(hi, sq_dst[:, cs], 1.0, None,
                                          ao.mult)
                        eng.tensor_tensor(lo, sq_dst[:, cs], hi, ao.subtract)
                    if sqkw2:
                        # sh2[c] = sq[c] (tail stays zero)
                        eng.tensor_scalar(
                            sq_f8[:, 4 * t + 2, c0:c1], sq_dst[:, cs],
                            1.0, None, ao.mult)
                if sqkw3:
                    # sh3[c] = sq[c+1] (tails stay zero)
                    nc.vector.tensor_scalar(
                        sq_f8[:, 4 * t + 3, 1:511],
                        planes["sq"][:, t * WPAD + 4: t * WPAD + 514],
                        1.0, None, ao.mult)
            emit_chans("sq")

            p_hi_last = H - (S * (NT - 1) - HALO)  # 96

            def edge_zero(nm):
                nc.gpsimd.memset(planes[nm][0:HALO, 0:WPAD], 0.0)
                nc.gpsimd.memset(
                    planes[nm][p_hi_last:P, (NT - 1) * WPAD: NT * WPAD], 0.0)

            # ---- conv1: bf16 banded matmuls from sq ----
            for o in range(2):
                psums = [
                    psum_pool.tile([P, W], F32, tag="ps", name=f"ps1_{o}_{t}")
                    for t in range(NT)
                ]
                for kw in range(4):
                    bandap = bands_sb[:, (NB_C1 + o * 4 + kw) * P:
                                      (NB_C1 + o * 4 + kw + 1) * P]
                    coff = 2 + kw * 1 - 1
                    for t in range(NT):
                        nc.tensor.matmul(
                            psums[t], bandap,
                            planes["sq"][:, t * WPAD + coff:
                                         t * WPAD + coff + W],
                            start=(kw == 0), stop=(kw == 3),
                        )
                bias_ap = bias_sb[:, o: o + 1]
                out_nm = f"c1_{o}"
                for t in range(NT):
                    nc.scalar.add(pslice(out_nm, t, 2, 514),
                                  psums[t][:], bias_ap)
                edge_zero(out_nm)
                emit_chans(out_nm)
                conv_to_pairs(out_nm)

            # ---- conv2: 3x3 dil2, DoubleRow (c1 pair + sq residual pair) ----
            for o in range(4):
                psums = [
                    psum_pool.tile([P, W], F32, tag="ps", name=f"ps2_{o}_{t}")
                    for t in range(NT)
                ]
                for kw in range(3):
                    lhsT = f8pair(F8_C2P + (o * 3 + kw) * 2)
                    coff = 2 + kw * 2 - 2
                    for t in range(NT):
                        rhs = fp8_pairs[6][:, 2 * t: 2 * t + 2,
                                           coff: coff + W]
                        nc.tensor.matmul(
                            psums[t], lhsT, rhs,
                            start=(kw == 0), stop=False, perf_mode=DR,
                        )
                if sqkw2:
                    # stream A: (kw0, kw1) via (hi, sh2) lanes at coff 0
                    lhsT = f8pair(F8_C2K + o * 2)
                    for t in range(NT):
                        rhs = sq_f8[:, 4 * t: 4 * t + 3: 2, 0:W]
                        nc.tensor.matmul(
                            psums[t], lhsT, rhs,
                            start=False, stop=False, perf_mode=DR,
                        )
                    # stream B: kw2 + residual via (hi, lo) lanes at coff 4
                    lhsT = f8pair(F8_C2S + (o * 3 + 2) * 2)
                    for t in range(NT):
                        rhs = sq_f8[:, 4 * t: 4 * t + 2, 4:4 + W]
                        nc.tensor.matmul(
                            psums[t], lhsT, rhs,
                            start=False, stop=True, perf_mode=DR,
                        )
                else:
                    for kw in range(3):
                        coff = 2 + kw * 2 - 2
                        if sq8:
                            lhsT = f8pair(F8_C2S + (o * 3 + kw) * 2)
                            for t in range(NT):
                                rhs = sq_f8[:, 4 * t: 4 * t + 2,
                                            coff: coff + W]
                                nc.tensor.matmul(
                                    psums[t], lhsT, rhs,
                                    start=False, stop=(kw == 2), perf_mode=DR,
                                )
                        else:
                            j = NB_C2S + o * 3 + kw
                            bandap = bands_sb[:, j * P:(j + 1) * P]
                            for t in range(NT):
                                nc.tensor.matmul(
                                    psums[t], bandap,
                                    planes["sq"][:, t * WPAD + coff:
                                                 t * WPAD + coff + W],
                                    start=False, stop=(kw == 2),
                                )
                bias_ap = bias_sb[:, 2 + o: 2 + o + 1]
                out_nm = f"c2_{o}"
                for t in range(NT):
                    nc.scalar.add(pslice(out_nm, t, 2, 514),
                                  psums[t][:], bias_ap)
                edge_zero(out_nm)
                emit_chans(out_nm)
                conv_to_pairs(out_nm)

            # conv4 DoubleRow pair-band table: diag(w) pairs. Built HERE (not
            # before conv1) so the 224 band ops don't sit ahead of conv2/3's
            # to_pair dependencies in the strict-FIFO engine queues; split
            # across DVE and GpSimd so neither engine stalls the pipeline.
            base4 = NSCAL - 240
            NPJ = len(PAIRS)
            ident_ap = bands_sb[:, nb_id * P:(nb_id + 1) * P]
            for o4 in range(16):
                eng = nc.vector if o4 % 2 == 0 else nc.gpsimd
                for pj, pr in enumerate(PAIRS):
                    for pi in (0, 1):
                        col = base4 + o4 * 15 + W4COL[pr[pi]]
                        k = ((o4 * NPJ + pj) * 2 + pi) * P
                        eng.tensor_scalar(
                            c4b_sb[:, k: k + P], ident_ap,
                            wtab_sb[:, col: col + 1], None, ao.mult,
                        )
                if not sq8:
                    colq = base4 + o4 * 15 + 14
                    eng.tensor_scalar(
                        sqb_sb[:, o4 * P:(o4 + 1) * P], ident_ap,
                        wtab_sb[:, colq: colq + 1], None, ao.mult,
                    )

            # ---- conv3: 2x2 dil3, DoubleRow; c3 evacs stream straight to
            # DRAM (+ fp8 pairs); no bf16 c3 planes are kept ----
            for o in range(8):
                psums = [
                    psum_pool.tile([P, W], F32, tag="ps", name=f"ps3_{o}_{t}")
                    for t in range(NT)
                ]
                first = True
                for pj3 in range(3):
                    for kw in range(2):
                        lhsT = f8pair(F8_C3P + ((o * 3 + pj3) * 2 + kw) * 2)
                        coff = 2 + kw * 3 - 1
                        for t in range(NT):
                            rhs = fp8_pairs[4 + pj3][:, 2 * t: 2 * t + 2,
                                                     coff: coff + W]
                            nc.tensor.matmul(
                                psums[t], lhsT, rhs,
                                start=first, stop=False, perf_mode=DR,
                            )
                        first = False
                if sqkw3:
                    # (kw0, kw1) in one stream via (hi, sh3) lanes at coff 1
                    lhsT = f8pair(F8_C3K + o * 2)
                    for t in range(NT):
                        rhs = sq_f8[:, 4 * t: 4 * t + 4: 3, 1:1 + W]
                        nc.tensor.matmul(
                            psums[t], lhsT, rhs,
                            start=False, stop=True, perf_mode=DR,
                        )
                else:
                    for kw in range(2):
                        coff = 2 + kw * 3 - 1
                        if sq8:
                            lhsT = f8pair(F8_C3S + (o * 2 + kw) * 2)
                            for t in range(NT):
                                rhs = sq_f8[:, 4 * t: 4 * t + 2,
                                            coff: coff + W]
                                nc.tensor.matmul(
                                    psums[t], lhsT, rhs,
                                    start=False, stop=(kw == 1), perf_mode=DR,
                                )
                        else:
                            j = NB_C3S + o * 2 + kw
                            bandap = bands_sb[:, j * P:(j + 1) * P]
                            for t in range(NT):
                                nc.tensor.matmul(
                                    psums[t], bandap,
                                    planes["sq"][:, t * WPAD + coff:
                                                 t * WPAD + coff + W],
                                    start=False, stop=(kw == 1),
                                )
                bias_ap = bias_sb[:, 6 + o: 6 + o + 1]
                # fp8 pairs for conv4 convert straight from PSUM (DVE does
                # bias-add + fp8 round) so conv4's inputs never wait on the
                # store scratch buffers or the out-DMA queue
                for t in range(NT):
                    for (pj, pi) in PAIR_OF[f"c3_{o}"]:
                        nc.vector.tensor_scalar(
                            fp8_pairs[pj][:, 2 * t + pi, 2:514],
                            psums[t][:], bias_ap, None, ao.add,
                        )
                st4 = c4_pool.tile([P, 4 * W], o_dt, tag="c4w")
                for t in range(4):
                    nc.scalar.add(st4[:, t * W:(t + 1) * W],
                                  psums[t][:], bias_ap)
                nc.sync.dma_start(
                    out=out_dram[16 + o, 0:4 * S, :].rearrange(
                        "(t r) w -> r t w", t=4),
                    in_=st4[HALO:HALO + S, :].rearrange(
                        "p (t w) -> p t w", t=4),
                )
                st = c4_pool.tile([P, W], o_dt, tag="c4")
                nc.scalar.add(st[:], psums[4][:], bias_ap)
                nc.sync.dma_start(
                    out=out_dram[16 + o, 4 * S:H, :],
                    in_=st[HALO:HALO + H - 4 * S, :],
                )

            # ---- conv4: 1x1, DoubleRow over 7 plane pairs + sq pair ----
            for o in range(16):
                psums = [
                    psum_pool.tile([P, W], F32, tag="ps", name=f"ps4_{o}_{t}")
                    for t in range(NT)
                ]
                for pj in range(NPJ):
                    k = (o * NPJ + pj) * 2 * P
                    lhsT = c4b_sb[:, k: k + 2 * P].rearrange(
                        "p (two m) -> p two m", two=2)
                    for t in range(NT):
                        rhs = fp8_pairs[pj][:, 2 * t: 2 * t + 2, 2:514]
                        nc.tensor.matmul(
                            psums[t], lhsT, rhs,
                            start=(pj == 0), stop=False, perf_mode=DR,
                        )
                if sq8:
                    lhsT = f8pair(F8_C4S + o * 2)
                    for t in range(NT):
                        rhs = sq_f8[:, 4 * t: 4 * t + 2, 2:514]
                        nc.tensor.matmul(
                            psums[t], lhsT, rhs,
                            start=False, stop=True, perf_mode=DR,
                        )
                else:
                    for t in range(NT):
                        nc.tensor.matmul(
                            psums[t], sqb_sb[:, o * P:(o + 1) * P],
                            planes["sq"][:, t * WPAD + 2: t * WPAD + 514],
                            start=False, stop=True,
                        )
                bias_ap = bias_sb[:, 14 + o: 14 + o + 1]
                if o == 15:
                    # last channel: per-tile stores with evacs spread across
                    # Act/DVE/GpSimd so the kernel tail is ~one evac+DMA long
                    for t in range(NT):
                        st = c4_pool.tile([P, W], o_dt, tag="c4")
                        if t % 2 == 0:
                            nc.scalar.add(st[:], psums[t][:], bias_ap)
                        else:
                            nc.vector.tensor_scalar(
                                st[:], psums[t][:], bias_ap, None, ao.add)
                        rows = S if t < NT - 1 else H - S * (NT - 1)
                        nc.sync.dma_start(
                            out=out_dram[o, S * t: S * t + rows, :],
                            in_=st[HALO:HALO + rows, :],
                        )
                    continue
                st4 = c4_pool.tile([P, 4 * W], o_dt, tag="c4w")
                for t in range(4):
                    nc.scalar.add(st4[:, t * W:(t + 1) * W],
                                  psums[t][:], bias_ap)
                nc.sync.dma_start(
                    out=out_dram[o, 0:4 * S, :].rearrange(
                        "(t r) w -> r t w", t=4),
                    in_=st4[HALO:HALO + S, :].rearrange(
                        "p (t w) -> p t w", t=4),
                )
                st = c4_pool.tile([P, W], o_dt, tag="c4")
                nc.scalar.add(st[:], psums[4][:], bias_ap)
                nc.sync.dma_start(
                    out=out_dram[o, 4 * S:H, :],
                    in_=st[HALO:HALO + H - 4 * S, :],
                )

    nc.compile()
    return nc


_NC_CACHE = None

KCFG = dict(sq8=True, sqkw='c3')


def _get_nc():
    global _NC_CACHE
    if _NC_CACHE is None:
        _NC_CACHE = build_nc(**KCFG)
    return _NC_CACHE


def _in_maps(inputs, n_cores):
    wtab, bias, bands, f8t = _host_tables(inputs)
    feat = inputs["feature_in"].astype(np.float32)  # [8,1,512,512]
    return [
        {"p": feat[b, 0], "wtab": wtab, "bias": bias,
         "bands": bands, "f8t": f8t}
        for b in range(n_cores)
    ]


def _run(inputs, trace=False):
    inputs = {k: np.asarray(v) for k, v in inputs.items()}
    nc = _get_nc()
    n_cores = inputs["feature_in"].shape[0]
    in_maps = _in_maps(inputs, n_cores)
    res = bass_utils.run_bass_kernel_spmd(
        nc, in_maps, core_ids=list(range(n_cores)), trace=trace
    )
    out = np.stack([res.results[b]["out"] for b in range(n_cores)], axis=0)
    return out.astype(np.float32), res


def kernel(**inputs):
    return _run(inputs, trace=False)[0]
